# revision 18
# baseline (speedup 1.0000x reference)
"""LocalVarianceNet Trainium2 kernel.

Computes E[x^2] - E[x]^2 over a 7x7 circular (wrap-padded) window, per
channel, for x of shape [16, 3, 512, 512] fp32.

Strategy (data parallel over 8 cores, 6 planes of 512x512 per core):
  Both separable box-filter passes run on the Tensor engine as banded
  matmuls. matmul(out, lhsT=data_chunk, rhs=B_band) computes
  data_chunk^T @ B_band: it filters the partition dim of the data while
  transposing it, so two passes compose back to natural orientation:
      pass1: Yt = X^T  B   (vertical sum over rows, output transposed)
      pass2: Z  = Yt^T B   (horizontal sum over cols, natural output)

  PSUM free-dim coordinates are rotated by +3 (c = i + 3 mod 512), which
  makes every 128-row chunk's band contribution a contiguous column
  range of ONE shared triangular band matrix Bband[kl, c] = 1 iff
  kl <= c <= kl+6 ([128, 134] incl. both wrap corners). 5 matmuls per
  512-col band pass. Intermediates are copied PSUM->SBUF into a 515-wide
  halo layout so pass-2 stationary slices stay contiguous.

  Host-side layout: the input is pre-permuted to [plane, q, kc, col]
  (q = row % 128, kc = row / 128) and pre-cast to fp16 so every inbound
  DMA is a straight contiguous HWDGE copy with 2KB+ lines. The output is
  written as fp16 in the same permuted layout, still carrying the +3
  column rotation; the host un-permutes, np.rolls by -3 and casts to
  fp32. This keeps all DMA packets large (the column rotation would
  otherwise fragment the output DMA into 12-byte packets).

  Engine split per plane: PE does all 16 band passes; Pool (no PSUM
  port) squares the input (fp16) and nothing else; the PSUM-touching
  copies/combines alternate between ACT and DVE.
"""

import numpy as np

P = 128
HW = 512
PAD = 3  # window 7 -> halo 3
NCH = 4  # 512 / 128 chunks
BW = P + 2 * PAD  # 134: band tile width
N_CORES = 8
PLANES_PER_CORE = 6  # (16 images * 3 channels) / 8 cores
N_WARM = 6  # junk matmuls to trip the PE HAM clock-gate during startup


def _make_bmat(np_dtype):
    """Triangular band tile [128, 134]: B[kl, c] = 1 iff kl <= c <= kl+6."""
    kl = np.arange(P)[:, None]
    c = np.arange(BW)[None, :]
    return np.ascontiguousarray(((kl <= c) & (c <= kl + 2 * PAD)).astype(np_dtype))


def _band_pass(nc, ps, lhsT_of, bm, sim_safe):
    """Circular 7-band filter into psum ps [128, 512] (rotated coords).

    ps[m, c] = sum_k lhsT_of(chunk(k))[kl, m] * B[k, (c - 3) mod 512]

    Chunk kc writes psum cols [128*kc, 128*kc + 134) (mod 512, the kc=3
    tail wraps to [0, 6)), always with rhs = the shared triangular band
    tile. sim_safe additionally splits the 6-col overlaps so every
    matmul's PSUM region is uniformly first-write or accumulate
    (CoreSim models has_written at instruction granularity).
    """
    OV = 2 * PAD  # 6-col overlap between adjacent chunk bands
    seq = []
    if sim_safe:
        seq.append((0, bm[:, 0:BW], ps[:, 0:BW], True))
        for kc in range(1, NCH):
            lo = kc * P
            w = BW if kc < NCH - 1 else P
            seq.append((kc, bm[:, 0:OV], ps[:, lo : lo + OV], False))
            seq.append((kc, bm[:, OV:w], ps[:, lo + OV : lo + w], False))
        seq.append((NCH - 1, bm[:, P:BW], ps[:, 0:OV], False))
    else:
        seq.append((0, bm[:, 0:BW], ps[:, 0:BW], True))
        for kc in range(1, NCH - 1):
            lo = kc * P
            seq.append((kc, bm[:, 0:BW], ps[:, lo : lo + BW], False))
        seq.append((NCH - 1, bm[:, 0:P], ps[:, (NCH - 1) * P : HW], False))
        seq.append((NCH - 1, bm[:, P:BW], ps[:, 0:OV], False))
    n = len(seq)
    for i, (kc, rh, out, start) in enumerate(seq):
        nc.tensor.matmul(out, lhsT_of(kc), rh, start=start, stop=(i == n - 1))


def build(n_planes=PLANES_PER_CORE, sim_safe=False):
    import concourse.mybir as mybir
    from concourse import bacc
    from concourse.tile import TileContext

    f16 = mybir.dt.float16
    f32 = mybir.dt.float32
    SQ = mybir.ActivationFunctionType.Square
    MUL = mybir.AluOpType.mult
    SUB = mybir.AluOpType.subtract
    INV = 1.0 / 49.0
    HB = HW + PAD  # 515: halo-extended width of the Yt tiles

    nc = bacc.Bacc("TRN2", target_bir_lowering=False)
    x_d = nc.declare_dram_parameter("x", [n_planes, P, NCH, HW], f16, isOutput=False)
    b_d = nc.declare_dram_parameter("bmat", [P, BW], f16, isOutput=False)
    o_d = nc.declare_dram_parameter("out", [n_planes, P, NCH, HW], f16, isOutput=True)

    with TileContext(nc) as tc:
        with (
            tc.tile_pool(name="const", bufs=1) as constp,
            tc.tile_pool(name="xin", bufs=3) as xinp,
            tc.tile_pool(name="xsq", bufs=3) as xsqp,
            tc.tile_pool(name="yt", bufs=2) as ytp,
            tc.tile_pool(name="tsq", bufs=4) as tsqp,
            tc.tile_pool(name="outp", bufs=3) as outpp,
            tc.tile_pool(name="psA", bufs=4, space="PSUM") as psAp,
            tc.tile_pool(name="psB", bufs=2, space="PSUM") as psBp,
        ):
            # tiny bm DMA first (the HWDGE ring is FIFO - putting the
            # 0.5MB plane-0 input ahead of it would delay every matmul),
            # plane-0 input second; both overlap the NEFF prologue.
            xins = [
                xinp.tile([P, NCH, HW], f16, name=f"xin{i}") for i in range(3)
            ]
            bm_t = constp.tile([P, BW], f16)
            nc.sync.dma_start(out=bm_t[:], in_=b_d[:, :])
            bm = bm_t[:]
            nc.sync.dma_start(out=xins[0][:], in_=x_d[0, :, :, :])

            # junk matmuls cover the PE HAM clock-gate ramp (~3.4us of
            # activity) while the plane-0 input DMA is still in flight
            junk = constp.tile([P, HW], f16)
            nc.vector.memset(junk[:], 0.0)
            warm = psAp.tile([P, HW], f32, tag="ps")
            for w in range(N_WARM):
                # junk is both operands: warmup only waits on the memset,
                # so it starts during the prologue, before any DMA lands
                nc.tensor.matmul(
                    warm[:], junk[:, 0:P], junk[:],
                    start=(w == 0), stop=(w == N_WARM - 1),
                )

            # --- software-pipelined schedule -----------------------------
            # Segment p emits pass2(p) zipped with pass1(p+1), so every
            # copy/combine dependency gets several band passes of PE cover
            # instead of less than one.
            xsqs = {}
            yts_all = {}

            def emit_sq(p):
                """squares for plane p on Pool, in column halves"""
                xin = xins[p]
                xsq = xsqp.tile([P, NCH, HW], f16, name="xsq")
                xsqs[p] = xsq
                for h in range(2):
                    sl = slice(h * HW // 2, (h + 1) * HW // 2)
                    nc.gpsimd.tensor_mul(
                        out=xsq[:, :, sl], in0=xin[:, :, sl], in1=xin[:, :, sl]
                    )

            def p1_cluster(p, k):
                """pass-1 cluster k (0-7) of plane p: band pass + copy"""
                t, jc = ("x", k) if k < NCH else ("x2", k - NCH)
                src = xins[p] if t == "x" else xsqs[p]
                if (t, p) not in yts_all:
                    yt = ytp.tile([P, NCH, HW], f16, tag=f"yt_{t}", name=f"yt_{t}")
                    yts_all[(t, p)] = yt
                yt = yts_all[(t, p)]
                ps = psAp.tile([P, HW], f32, tag="ps", name="psA_t")
                _band_pass(
                    nc, ps[:],
                    lambda kc: src[:, kc, jc * P : (jc + 1) * P],
                    bm, sim_safe,
                )
                if jc % 2 == 0:
                    nc.scalar.copy(out=yt[:, jc, :], in_=ps[:])
                else:
                    nc.vector.tensor_copy(out=yt[:, jc, :], in_=ps[:])

            P2_ORDER = (
                (0, "x"), (1, "x"), (0, "x2"), (1, "x2"),
                (2, "x"), (3, "x"), (2, "x2"), (3, "x2"),
            )

            def p2_cluster(p, k, outt, tss):
                """pass-2 cluster k (0-7) of plane p: band pass + combine"""
                ic, t = P2_ORDER[k]
                lo = ic * P
                if t == "x":
                    ps1 = psBp.tile([P, HW], f32, tag="s1", name="ps1_t")
                    _band_pass(
                        nc, ps1[:],
                        lambda jc: yts_all[("x", p)][:, jc, lo : lo + P],
                        bm, sim_safe,
                    )
                    ts_ = tsqp.tile([P, HW], f32, name="ts_t")
                    tss[ic] = ts_
                    nc.scalar.activation(out=ts_[:], in_=ps1[:], func=SQ, scale=INV)
                else:
                    ps2 = psBp.tile([P, HW], f32, tag="s2", name="ps2_t")
                    _band_pass(
                        nc, ps2[:],
                        lambda jc: yts_all[("x2", p)][:, jc, lo : lo + P],
                        bm, sim_safe,
                    )
                    nc.vector.scalar_tensor_tensor(
                        out=outt[:, ic, :], in0=ps2[:], scalar=INV,
                        in1=tss[ic][:], op0=MUL, op1=SUB,
                    )
                    # output stays +3-rotated (rows AND cols) and
                    # [q, ic, c]-permuted fp16; host fixes all of it.
                    if p < n_planes - 1:
                        if ic == 1:
                            nc.sync.dma_start(
                                out=o_d[p, :, 0:2, :], in_=outt[:, 0:2, :]
                            )
                        elif ic == 3:
                            nc.sync.dma_start(
                                out=o_d[p, :, 2:4, :], in_=outt[:, 2:4, :]
                            )
                    else:  # last plane drains per-ic: shorter tail
                        sl = slice(ic, ic + 1)
                        nc.sync.dma_start(out=o_d[p, :, sl, :], in_=outt[:, sl, :])

            # prologue: squares + full pass 1 of plane 0. Inputs for
            # planes 1-2 are dispatched after plane 0's (same SDMA rings
            # round-robin at packet granularity - dispatching them earlier
            # would slow plane 0's transfer down).
            emit_sq(0)
            for k in range(2):
                p1_cluster(0, k)
            nc.sync.dma_start(out=xins[1][:], in_=x_d[1, :, :, :])
            nc.sync.dma_start(out=xins[2][:], in_=x_d[2, :, :, :])
            for k in range(2, 8):
                p1_cluster(0, k)

            for p in range(n_planes):
                # prefetch input for plane p+3 and squares for plane p+1
                if p + 3 < n_planes:
                    xin_n = xinp.tile([P, NCH, HW], f16, name="xin_t")
                    xins.append(xin_n)
                    nc.sync.dma_start(out=xin_n[:], in_=x_d[p + 3, :, :, :])
                if p + 1 < n_planes:
                    emit_sq(p + 1)
                outt = outpp.tile([P, NCH, HW], f16, name="outt")
                tss = {}
                for k in range(8):
                    p2_cluster(p, k, outt, tss)
                    if p + 1 < n_planes:
                        p1_cluster(p + 1, k)
    nc.compile()
    return nc


_CACHED = {}


def _get_nc(n_planes=PLANES_PER_CORE):
    if n_planes not in _CACHED:
        _CACHED[n_planes] = build(n_planes)
    return _CACHED[n_planes]


def kernel(x: np.ndarray) -> np.ndarray:
    from concourse.bass_utils import run_bass_kernel_spmd

    N, C, H, W = x.shape
    assert (H, W) == (HW, HW), (H, W)
    total = N * C
    per_core = total // N_CORES
    assert per_core == PLANES_PER_CORE, (total, N_CORES)

    # host-side permute + cast: xp[p, q, kc, c] = x[p, kc*128+q, c] as fp16
    planes = x.reshape(total, H, W)
    xp = np.ascontiguousarray(
        planes.reshape(total, NCH, P, HW).transpose(0, 2, 1, 3).astype(np.float16)
    )

    bmat = _make_bmat(np.float16)
    nc = _get_nc(per_core)

    in_maps = [
        {
            "x": np.ascontiguousarray(xp[i * per_core : (i + 1) * per_core]),
            "bmat": bmat,
        }
        for i in range(N_CORES)
    ]
    res = run_bass_kernel_spmd(nc, in_maps, list(range(N_CORES)))
    out = np.concatenate([r["out"] for r in res.results], axis=0)
    # out[p, q, ic, c] = var[p, (ic*128+q-3)%512, (c-3)%512] in fp16
    # (pass-2 slices start at ic*128, so rows carry a +3 rotation too)
    o = out.transpose(0, 2, 1, 3).reshape(total, HW, HW)
    o = np.roll(o, -3, axis=(1, 2))
    return np.ascontiguousarray(o.reshape(N, C, H, W).astype(np.float32))


# revision 20
# speedup vs baseline: 1.1225x; 1.1225x over previous
"""LocalVarianceNet Trainium2 kernel.

Computes E[x^2] - E[x]^2 over a 7x7 circular (wrap-padded) window, per
channel, for x of shape [16, 3, 512, 512] fp32.

Strategy (data parallel over 8 cores, 6 planes of 512x512 per core):
  Both separable box-filter passes run on the Tensor engine as banded
  matmuls. matmul(out, lhsT=data_chunk, rhs=B_band) computes
  data_chunk^T @ B_band: it filters the partition dim of the data while
  transposing it, so two passes compose back to natural orientation:
      pass1: Yt = X^T  B   (vertical sum over rows, output transposed)
      pass2: Z  = Yt^T B   (horizontal sum over cols, natural output)

  PSUM free-dim coordinates are rotated by +3 (c = i + 3 mod 512), which
  makes every 128-row chunk's band contribution a contiguous column
  range of ONE shared triangular band matrix Bband[kl, c] = 1 iff
  kl <= c <= kl+6 ([128, 134] incl. both wrap corners). 5 matmuls per
  512-col band pass. Intermediates are copied PSUM->SBUF into a 515-wide
  halo layout so pass-2 stationary slices stay contiguous.

  Host-side layout: the input is pre-permuted to [plane, q, kc, col]
  (q = row % 128, kc = row / 128) and pre-cast to fp16 so every inbound
  DMA is a straight contiguous HWDGE copy with 2KB+ lines. The output is
  written as fp16 in the same permuted layout, still carrying the +3
  column rotation; the host un-permutes, np.rolls by -3 and casts to
  fp32. This keeps all DMA packets large (the column rotation would
  otherwise fragment the output DMA into 12-byte packets).

  Engine split per plane: PE does all 16 band passes; Pool (no PSUM
  port) squares the input (fp16) and nothing else; the PSUM-touching
  copies/combines alternate between ACT and DVE.
"""

import numpy as np

P = 128
HW = 512
PAD = 3  # window 7 -> halo 3
NCH = 4  # 512 / 128 chunks
BW = P + 2 * PAD  # 134: band tile width
N_CORES = 8
PLANES_PER_CORE = 6  # (16 images * 3 channels) / 8 cores
N_WARM = 6  # junk matmuls to trip the PE HAM clock-gate during startup


def _make_bmat(np_dtype):
    """Triangular band tile [128, 134]: B[kl, c] = 1 iff kl <= c <= kl+6."""
    kl = np.arange(P)[:, None]
    c = np.arange(BW)[None, :]
    return np.ascontiguousarray(((kl <= c) & (c <= kl + 2 * PAD)).astype(np_dtype))


def _band_pass(nc, ps, lhsT_of, bm, sim_safe):
    """Circular 7-band filter into psum ps [128, 512] (rotated coords).

    ps[m, c] = sum_k lhsT_of(chunk(k))[kl, m] * B[k, (c - 3) mod 512]

    Chunk kc writes psum cols [128*kc, 128*kc + 134) (mod 512, the kc=3
    tail wraps to [0, 6)), always with rhs = the shared triangular band
    tile. sim_safe additionally splits the 6-col overlaps so every
    matmul's PSUM region is uniformly first-write or accumulate
    (CoreSim models has_written at instruction granularity).
    """
    OV = 2 * PAD  # 6-col overlap between adjacent chunk bands
    seq = []
    if sim_safe:
        seq.append((0, bm[:, 0:BW], ps[:, 0:BW], True))
        for kc in range(1, NCH):
            lo = kc * P
            w = BW if kc < NCH - 1 else P
            seq.append((kc, bm[:, 0:OV], ps[:, lo : lo + OV], False))
            seq.append((kc, bm[:, OV:w], ps[:, lo + OV : lo + w], False))
        seq.append((NCH - 1, bm[:, P:BW], ps[:, 0:OV], False))
    else:
        seq.append((0, bm[:, 0:BW], ps[:, 0:BW], True))
        for kc in range(1, NCH - 1):
            lo = kc * P
            seq.append((kc, bm[:, 0:BW], ps[:, lo : lo + BW], False))
        seq.append((NCH - 1, bm[:, 0:P], ps[:, (NCH - 1) * P : HW], False))
        seq.append((NCH - 1, bm[:, P:BW], ps[:, 0:OV], False))
    n = len(seq)
    for i, (kc, rh, out, start) in enumerate(seq):
        nc.tensor.matmul(out, lhsT_of(kc), rh, start=start, stop=(i == n - 1))


def build(n_planes=PLANES_PER_CORE, sim_safe=False):
    import concourse.mybir as mybir
    from concourse import bacc
    from concourse.tile import TileContext

    f16 = mybir.dt.float16
    f32 = mybir.dt.float32
    SQ = mybir.ActivationFunctionType.Square
    MUL = mybir.AluOpType.mult
    SUB = mybir.AluOpType.subtract
    INV = 1.0 / 49.0
    HB = HW + PAD  # 515: halo-extended width of the Yt tiles

    nc = bacc.Bacc("TRN2", target_bir_lowering=False)
    x_d = nc.declare_dram_parameter("x", [n_planes, P, NCH, HW], f16, isOutput=False)
    b_d = nc.declare_dram_parameter("bmat", [P, BW], f16, isOutput=False)
    o_d = nc.declare_dram_parameter("out", [n_planes, P, NCH, HW], f16, isOutput=True)

    with TileContext(nc) as tc:
        with (
            tc.tile_pool(name="const", bufs=1) as constp,
            tc.tile_pool(name="xin", bufs=3) as xinp,
            tc.tile_pool(name="xsq", bufs=3) as xsqp,
            tc.tile_pool(name="yt", bufs=2) as ytp,
            tc.tile_pool(name="tsq", bufs=4) as tsqp,
            tc.tile_pool(name="outp", bufs=3) as outpp,
            tc.tile_pool(name="psA", bufs=4, space="PSUM") as psAp,
            tc.tile_pool(name="psB", bufs=2, space="PSUM") as psBp,
        ):
            # tiny bm DMA first (the HWDGE ring is FIFO - putting the
            # 0.5MB plane-0 input ahead of it would delay every matmul),
            # plane-0 input second; both overlap the NEFF prologue.
            xins = [
                xinp.tile([P, NCH, HW], f16, name=f"xin{i}") for i in range(3)
            ]
            bm_t = constp.tile([P, BW], f16)
            nc.sync.dma_start(out=bm_t[:], in_=b_d[:, :])
            bm = bm_t[:]
            nc.sync.dma_start(out=xins[0][:], in_=x_d[0, :, :, :])

            # junk matmuls cover the PE HAM clock-gate ramp (~3.4us of
            # activity) while the plane-0 input DMA is still in flight
            junk = constp.tile([P, HW], f16)
            nc.vector.memset(junk[:], 0.0)
            warm = psAp.tile([P, HW], f32, tag="ps")
            for w in range(N_WARM):
                # junk is both operands: warmup only waits on the memset,
                # so it starts during the prologue, before any DMA lands
                nc.tensor.matmul(
                    warm[:, 0:BW], junk[:, 0:P], junk[:, 0:BW],
                    start=(w == 0), stop=(w == N_WARM - 1),
                )

            # --- software-pipelined schedule -----------------------------
            # Segment p emits pass2(p) zipped with pass1(p+1), so every
            # copy/combine dependency gets several band passes of PE cover
            # instead of less than one.
            xsqs = {}
            yts_all = {}

            def emit_sq(p):
                """squares for plane p on Pool, in column halves"""
                xin = xins[p]
                xsq = xsqp.tile([P, NCH, HW], f16, name="xsq")
                xsqs[p] = xsq
                for h in range(2):
                    sl = slice(h * HW // 2, (h + 1) * HW // 2)
                    nc.gpsimd.tensor_mul(
                        out=xsq[:, :, sl], in0=xin[:, :, sl], in1=xin[:, :, sl]
                    )

            def p1_cluster(p, k):
                """pass-1 cluster k (0-7) of plane p: band pass + copy"""
                t, jc = ("x", k) if k < NCH else ("x2", k - NCH)
                src = xins[p] if t == "x" else xsqs[p]
                if (t, p) not in yts_all:
                    yt = ytp.tile([P, NCH, HW], f16, tag=f"yt_{t}", name=f"yt_{t}")
                    yts_all[(t, p)] = yt
                yt = yts_all[(t, p)]
                ps = psAp.tile([P, HW], f32, tag="ps", name="psA_t")
                _band_pass(
                    nc, ps[:],
                    lambda kc: src[:, kc, jc * P : (jc + 1) * P],
                    bm, sim_safe,
                )
                if jc % 2 == 0:
                    nc.scalar.copy(out=yt[:, jc, :], in_=ps[:])
                else:
                    nc.vector.tensor_copy(out=yt[:, jc, :], in_=ps[:])

            P2_ORDER = (
                (0, "x"), (1, "x"), (0, "x2"), (1, "x2"),
                (2, "x"), (3, "x"), (2, "x2"), (3, "x2"),
            )

            def p2_cluster(p, k, outt, tss):
                """pass-2 cluster k (0-7) of plane p: band pass + combine"""
                ic, t = P2_ORDER[k]
                lo = ic * P
                if t == "x":
                    ps1 = psBp.tile([P, HW], f32, tag="s1", name="ps1_t")
                    _band_pass(
                        nc, ps1[:],
                        lambda jc: yts_all[("x", p)][:, jc, lo : lo + P],
                        bm, sim_safe,
                    )
                    ts_ = tsqp.tile([P, HW], f32, name="ts_t")
                    tss[ic] = ts_
                    nc.scalar.activation(out=ts_[:], in_=ps1[:], func=SQ, scale=INV)
                else:
                    ps2 = psBp.tile([P, HW], f32, tag="s2", name="ps2_t")
                    _band_pass(
                        nc, ps2[:],
                        lambda jc: yts_all[("x2", p)][:, jc, lo : lo + P],
                        bm, sim_safe,
                    )
                    nc.vector.scalar_tensor_tensor(
                        out=outt[:, ic, :], in0=ps2[:], scalar=INV,
                        in1=tss[ic][:], op0=MUL, op1=SUB,
                    )
                    # output stays +3-rotated (rows AND cols) and
                    # [q, ic, c]-permuted fp16; host fixes all of it.
                    if p < n_planes - 1:
                        if ic == 1:
                            nc.sync.dma_start(
                                out=o_d[p, :, 0:2, :], in_=outt[:, 0:2, :]
                            )
                        elif ic == 3:
                            nc.sync.dma_start(
                                out=o_d[p, :, 2:4, :], in_=outt[:, 2:4, :]
                            )
                    else:  # last plane drains per-ic: shorter tail
                        sl = slice(ic, ic + 1)
                        nc.sync.dma_start(out=o_d[p, :, sl, :], in_=outt[:, sl, :])

            # prologue: squares + full pass 1 of plane 0. Inputs for
            # planes 1-2 are dispatched after plane 0's (same SDMA rings
            # round-robin at packet granularity - dispatching them earlier
            # would slow plane 0's transfer down).
            emit_sq(0)
            for k in range(2):
                p1_cluster(0, k)
            nc.sync.dma_start(out=xins[1][:], in_=x_d[1, :, :, :])
            nc.sync.dma_start(out=xins[2][:], in_=x_d[2, :, :, :])
            for k in range(2, 8):
                p1_cluster(0, k)

            for p in range(n_planes):
                # prefetch input for plane p+3 and squares for plane p+1
                if p + 3 < n_planes:
                    xin_n = xinp.tile([P, NCH, HW], f16, name="xin_t")
                    xins.append(xin_n)
                    nc.sync.dma_start(out=xin_n[:], in_=x_d[p + 3, :, :, :])
                if p + 1 < n_planes:
                    emit_sq(p + 1)
                outt = outpp.tile([P, NCH, HW], f16, name="outt")
                tss = {}
                for k in range(8):
                    p2_cluster(p, k, outt, tss)
                    if p + 1 < n_planes:
                        p1_cluster(p + 1, k)
    nc.compile()
    return nc


_CACHED = {}


def _get_nc(n_planes=PLANES_PER_CORE):
    if n_planes not in _CACHED:
        _CACHED[n_planes] = build(n_planes)
    return _CACHED[n_planes]


def kernel(x: np.ndarray) -> np.ndarray:
    from concourse.bass_utils import run_bass_kernel_spmd

    N, C, H, W = x.shape
    assert (H, W) == (HW, HW), (H, W)
    total = N * C
    per_core = total // N_CORES
    assert per_core == PLANES_PER_CORE, (total, N_CORES)

    # host-side permute + cast: xp[p, q, kc, c] = x[p, kc*128+q, c] as fp16
    planes = x.reshape(total, H, W)
    xp = np.ascontiguousarray(
        planes.reshape(total, NCH, P, HW).transpose(0, 2, 1, 3).astype(np.float16)
    )

    bmat = _make_bmat(np.float16)
    nc = _get_nc(per_core)

    in_maps = [
        {
            "x": np.ascontiguousarray(xp[i * per_core : (i + 1) * per_core]),
            "bmat": bmat,
        }
        for i in range(N_CORES)
    ]
    res = run_bass_kernel_spmd(nc, in_maps, list(range(N_CORES)))
    out = np.concatenate([r["out"] for r in res.results], axis=0)
    # out[p, q, ic, c] = var[p, (ic*128+q-3)%512, (c-3)%512] in fp16
    # (pass-2 slices start at ic*128, so rows carry a +3 rotation too)
    o = out.transpose(0, 2, 1, 3).reshape(total, HW, HW)
    o = np.roll(o, -3, axis=(1, 2))
    return np.ascontiguousarray(o.reshape(N, C, H, W).astype(np.float32))


# revision 21
# speedup vs baseline: 1.1754x; 1.0471x over previous
"""LocalVarianceNet Trainium2 kernel.

Computes E[x^2] - E[x]^2 over a 7x7 circular (wrap-padded) window, per
channel, for x of shape [16, 3, 512, 512] fp32.

Strategy (data parallel over 8 cores, 6 planes of 512x512 per core):
  Both separable box-filter passes run on the Tensor engine as banded
  matmuls. matmul(out, lhsT=data_chunk, rhs=B_band) computes
  data_chunk^T @ B_band: it filters the partition dim of the data while
  transposing it, so two passes compose back to natural orientation:
      pass1: Yt = X^T  B   (vertical sum over rows, output transposed)
      pass2: Z  = Yt^T B   (horizontal sum over cols, natural output)

  PSUM free-dim coordinates are rotated by +3 (c = i + 3 mod 512), which
  makes every 128-row chunk's band contribution a contiguous column
  range of ONE shared triangular band matrix Bband[kl, c] = 1 iff
  kl <= c <= kl+6 ([128, 134] incl. both wrap corners). 5 matmuls per
  512-col band pass. Intermediates are copied PSUM->SBUF into a 515-wide
  halo layout so pass-2 stationary slices stay contiguous.

  Host-side layout: the input is pre-permuted to [plane, q, kc, col]
  (q = row % 128, kc = row / 128) and pre-cast to fp16 so every inbound
  DMA is a straight contiguous HWDGE copy with 2KB+ lines. The output is
  written as fp16 in the same permuted layout, still carrying the +3
  column rotation; the host un-permutes, np.rolls by -3 and casts to
  fp32. This keeps all DMA packets large (the column rotation would
  otherwise fragment the output DMA into 12-byte packets).

  Engine split per plane: PE does all 16 band passes; Pool (no PSUM
  port) squares the input (fp16) and nothing else; the PSUM-touching
  copies/combines alternate between ACT and DVE.
"""

import numpy as np

P = 128
HW = 512
PAD = 3  # window 7 -> halo 3
NCH = 4  # 512 / 128 chunks
BW = P + 2 * PAD  # 134: band tile width
N_CORES = 8
PLANES_PER_CORE = 6  # (16 images * 3 channels) / 8 cores
N_WARM = 3  # junk matmuls to trip the PE HAM clock-gate during startup


def _make_bmat(np_dtype):
    """Triangular band tile [128, 134]: B[kl, c] = 1 iff kl <= c <= kl+6."""
    kl = np.arange(P)[:, None]
    c = np.arange(BW)[None, :]
    return np.ascontiguousarray(((kl <= c) & (c <= kl + 2 * PAD)).astype(np_dtype))


def _band_pass(nc, ps, lhsT_of, bm, sim_safe):
    """Circular 7-band filter into psum ps [128, 512] (rotated coords).

    ps[m, c] = sum_k lhsT_of(chunk(k))[kl, m] * B[k, (c - 3) mod 512]

    Chunk kc writes psum cols [128*kc, 128*kc + 134) (mod 512, the kc=3
    tail wraps to [0, 6)), always with rhs = the shared triangular band
    tile. sim_safe additionally splits the 6-col overlaps so every
    matmul's PSUM region is uniformly first-write or accumulate
    (CoreSim models has_written at instruction granularity).
    """
    OV = 2 * PAD  # 6-col overlap between adjacent chunk bands
    seq = []
    if sim_safe:
        seq.append((0, bm[:, 0:BW], ps[:, 0:BW], True))
        for kc in range(1, NCH):
            lo = kc * P
            w = BW if kc < NCH - 1 else P
            seq.append((kc, bm[:, 0:OV], ps[:, lo : lo + OV], False))
            seq.append((kc, bm[:, OV:w], ps[:, lo + OV : lo + w], False))
        seq.append((NCH - 1, bm[:, P:BW], ps[:, 0:OV], False))
    else:
        seq.append((0, bm[:, 0:BW], ps[:, 0:BW], True))
        for kc in range(1, NCH - 1):
            lo = kc * P
            seq.append((kc, bm[:, 0:BW], ps[:, lo : lo + BW], False))
        seq.append((NCH - 1, bm[:, 0:P], ps[:, (NCH - 1) * P : HW], False))
        seq.append((NCH - 1, bm[:, P:BW], ps[:, 0:OV], False))
    n = len(seq)
    for i, (kc, rh, out, start) in enumerate(seq):
        nc.tensor.matmul(out, lhsT_of(kc), rh, start=start, stop=(i == n - 1))


def build(n_planes=PLANES_PER_CORE, sim_safe=False):
    import concourse.mybir as mybir
    from concourse import bacc
    from concourse.tile import TileContext

    f16 = mybir.dt.float16
    f32 = mybir.dt.float32
    SQ = mybir.ActivationFunctionType.Square
    MUL = mybir.AluOpType.mult
    SUB = mybir.AluOpType.subtract
    INV = 1.0 / 49.0
    HB = HW + PAD  # 515: halo-extended width of the Yt tiles

    nc = bacc.Bacc("TRN2", target_bir_lowering=False)
    x_d = nc.declare_dram_parameter("x", [n_planes, P, NCH, HW], f16, isOutput=False)
    b_d = nc.declare_dram_parameter("bmat", [P, BW], f16, isOutput=False)
    o_d = nc.declare_dram_parameter("out", [n_planes, P, NCH, HW], f16, isOutput=True)

    with TileContext(nc) as tc:
        with (
            tc.tile_pool(name="const", bufs=1) as constp,
            tc.tile_pool(name="xin", bufs=3) as xinp,
            tc.tile_pool(name="xsq", bufs=3) as xsqp,
            tc.tile_pool(name="yt", bufs=2) as ytp,
            tc.tile_pool(name="tsq", bufs=4) as tsqp,
            tc.tile_pool(name="outp", bufs=3) as outpp,
            tc.tile_pool(name="psA", bufs=4, space="PSUM") as psAp,
            tc.tile_pool(name="psB", bufs=2, space="PSUM") as psBp,
        ):
            # plane-0 input DMA first, tiny bm DMA second; both overlap
            # the NEFF prologue (HWDGE ring is FIFO).
            xins = [
                xinp.tile([P, NCH, HW], f16, name=f"xin{i}") for i in range(3)
            ]
            nc.sync.dma_start(out=xins[0][:], in_=x_d[0, :, :, :])
            bm_t = constp.tile([P, BW], f16)
            nc.sync.dma_start(out=bm_t[:], in_=b_d[:, :])
            bm = bm_t[:]

            # junk matmuls cover the PE HAM clock-gate ramp (~3.4us of
            # activity) while the plane-0 input DMA is still in flight
            junk = constp.tile([P, HW], f16)
            nc.vector.memset(junk[:], 0.0)
            warm = psAp.tile([P, HW], f32, tag="ps")
            for w in range(N_WARM):
                nc.tensor.matmul(
                    warm[:], bm[:, 0:P], junk[:],
                    start=(w == 0), stop=(w == N_WARM - 1),
                )

            # --- software-pipelined schedule -----------------------------
            # Segment p emits pass2(p) zipped with pass1(p+1), so every
            # copy/combine dependency gets several band passes of PE cover
            # instead of less than one.
            xsqs = {}
            yts_all = {}

            def emit_sq(p):
                """squares for plane p on Pool, in column halves"""
                xin = xins[p]
                xsq = xsqp.tile([P, NCH, HW], f16, name="xsq")
                xsqs[p] = xsq
                for h in range(2):
                    sl = slice(h * HW // 2, (h + 1) * HW // 2)
                    nc.gpsimd.tensor_mul(
                        out=xsq[:, :, sl], in0=xin[:, :, sl], in1=xin[:, :, sl]
                    )

            def p1_cluster(p, k):
                """pass-1 cluster k (0-7) of plane p: band pass + copy"""
                t, jc = ("x", k) if k < NCH else ("x2", k - NCH)
                src = xins[p] if t == "x" else xsqs[p]
                if (t, p) not in yts_all:
                    yt = ytp.tile([P, NCH, HW], f16, tag=f"yt_{t}", name=f"yt_{t}")
                    yts_all[(t, p)] = yt
                yt = yts_all[(t, p)]
                ps = psAp.tile([P, HW], f32, tag="ps", name="psA_t")
                _band_pass(
                    nc, ps[:],
                    lambda kc: src[:, kc, jc * P : (jc + 1) * P],
                    bm, sim_safe,
                )
                if jc % 2 == 0:
                    nc.scalar.copy(out=yt[:, jc, :], in_=ps[:])
                else:
                    nc.vector.tensor_copy(out=yt[:, jc, :], in_=ps[:])

            P2_ORDER = (
                (0, "x"), (1, "x"), (0, "x2"), (1, "x2"),
                (2, "x"), (3, "x"), (2, "x2"), (3, "x2"),
            )

            def p2_cluster(p, k, outt, tss):
                """pass-2 cluster k (0-7) of plane p: band pass + combine"""
                ic, t = P2_ORDER[k]
                lo = ic * P
                if t == "x":
                    ps1 = psBp.tile([P, HW], f32, tag="s1", name="ps1_t")
                    _band_pass(
                        nc, ps1[:],
                        lambda jc: yts_all[("x", p)][:, jc, lo : lo + P],
                        bm, sim_safe,
                    )
                    ts_ = tsqp.tile([P, HW], f32, name="ts_t")
                    tss[ic] = ts_
                    nc.scalar.activation(out=ts_[:], in_=ps1[:], func=SQ, scale=INV)
                else:
                    ps2 = psBp.tile([P, HW], f32, tag="s2", name="ps2_t")
                    _band_pass(
                        nc, ps2[:],
                        lambda jc: yts_all[("x2", p)][:, jc, lo : lo + P],
                        bm, sim_safe,
                    )
                    nc.vector.scalar_tensor_tensor(
                        out=outt[:, ic, :], in0=ps2[:], scalar=INV,
                        in1=tss[ic][:], op0=MUL, op1=SUB,
                    )
                    # output stays +3-rotated (rows AND cols) and
                    # [q, ic, c]-permuted fp16; host fixes all of it.
                    if p < n_planes - 1:
                        if ic == 1:
                            nc.sync.dma_start(
                                out=o_d[p, :, 0:2, :], in_=outt[:, 0:2, :]
                            )
                        elif ic == 3:
                            nc.sync.dma_start(
                                out=o_d[p, :, 2:4, :], in_=outt[:, 2:4, :]
                            )
                    else:  # last plane drains per-ic: shorter tail
                        sl = slice(ic, ic + 1)
                        nc.sync.dma_start(out=o_d[p, :, sl, :], in_=outt[:, sl, :])

            # prologue: squares + full pass 1 of plane 0. Inputs for
            # planes 1-2 are dispatched after plane 0's (same SDMA rings
            # round-robin at packet granularity - dispatching them earlier
            # would slow plane 0's transfer down).
            emit_sq(0)
            for k in range(2):
                p1_cluster(0, k)
            nc.sync.dma_start(out=xins[1][:], in_=x_d[1, :, :, :])
            nc.sync.dma_start(out=xins[2][:], in_=x_d[2, :, :, :])
            for k in range(2, 8):
                p1_cluster(0, k)

            for p in range(n_planes):
                # prefetch input for plane p+3 and squares for plane p+1
                if p + 3 < n_planes:
                    xin_n = xinp.tile([P, NCH, HW], f16, name="xin_t")
                    xins.append(xin_n)
                    nc.sync.dma_start(out=xin_n[:], in_=x_d[p + 3, :, :, :])
                if p + 1 < n_planes:
                    emit_sq(p + 1)
                outt = outpp.tile([P, NCH, HW], f16, name="outt")
                tss = {}
                for k in range(8):
                    p2_cluster(p, k, outt, tss)
                    if p + 1 < n_planes:
                        p1_cluster(p + 1, k)
    nc.compile()
    return nc


_CACHED = {}


def _get_nc(n_planes=PLANES_PER_CORE):
    if n_planes not in _CACHED:
        _CACHED[n_planes] = build(n_planes)
    return _CACHED[n_planes]


def kernel(x: np.ndarray) -> np.ndarray:
    from concourse.bass_utils import run_bass_kernel_spmd

    N, C, H, W = x.shape
    assert (H, W) == (HW, HW), (H, W)
    total = N * C
    per_core = total // N_CORES
    assert per_core == PLANES_PER_CORE, (total, N_CORES)

    # host-side permute + cast: xp[p, q, kc, c] = x[p, kc*128+q, c] as fp16
    planes = x.reshape(total, H, W)
    xp = np.ascontiguousarray(
        planes.reshape(total, NCH, P, HW).transpose(0, 2, 1, 3).astype(np.float16)
    )

    bmat = _make_bmat(np.float16)
    nc = _get_nc(per_core)

    in_maps = [
        {
            "x": np.ascontiguousarray(xp[i * per_core : (i + 1) * per_core]),
            "bmat": bmat,
        }
        for i in range(N_CORES)
    ]
    res = run_bass_kernel_spmd(nc, in_maps, list(range(N_CORES)))
    out = np.concatenate([r["out"] for r in res.results], axis=0)
    # out[p, q, ic, c] = var[p, (ic*128+q-3)%512, (c-3)%512] in fp16
    # (pass-2 slices start at ic*128, so rows carry a +3 rotation too)
    o = out.transpose(0, 2, 1, 3).reshape(total, HW, HW)
    o = np.roll(o, -3, axis=(1, 2))
    return np.ascontiguousarray(o.reshape(N, C, H, W).astype(np.float32))
